# revision 5
# baseline (speedup 1.0000x reference)
"""Trainium2 Bass kernel for nn_Basenet_collective (ragged RoI-box MLP head).

Computes, for X=[80,13,26400] box features with ragged per-frame validity:
    h = relu(X @ w_emb + b_emb)                      [80,13,1024]
    actions = (h @ w_act + b_act) * valid_mask       [80,13,6]
    activities = max_pool_valid(h) @ w_acty + b_acty [80,5]

Distribution over 8 NeuronCores (one trn2 chip):
  - Host compacts the ragged box axis (only sum(bboxes_num) of the 80*13
    box slots contribute to the output), groups frames by box count, and
    transposes X to [K2D, V].
  - The 26400-deep contraction is split 8 ways (3300 per core); each core
    computes a partial H^T = w_emb_slice^T @ X_slice^T of shape [1024, V]
    on the tensor engine (float32r, fp32 PSUM accumulation).
  - A ReduceScatter(add) leaves core i with feature rows [128i:128(i+1)]
    of the summed H^T; stage 2 (bias+relu, action scores, grouped
    max-pool, activity scores) is feature-sharded and frame-local.
  - Host sums the 8 feature-shard partials of the two small outputs,
    adds biases, and scatters back to the original ragged layout.
"""

import numpy as np

_BT = 80
_MAXN = 13
_K2D = 26400
_NFB = 1024
_A = 6
_G = 5
_NC = 8
_KC = _K2D // _NC   # 3300 contraction rows per core
_FC = _NFB // _NC   # 128 feature rows per core after ReduceScatter

# 'f32r' (full-rate fp32, near-fp32 precision), 'bf16', or 'f32' (4x slower)
_MM_DTYPE = "f32r"


def _plan(bboxes_num):
    """Host-side plan: compaction order, pooling groups, column chunks."""
    n = np.asarray(bboxes_num).astype(np.int64)
    assert n.shape == (_BT,) and n.min() >= 1 and n.max() <= _MAXN
    order = np.argsort(n, kind="stable")          # frames sorted by box count
    ns = n[order]
    V = int(n.sum())
    Vp = ((max(V, 64) + 63) // 64) * 64           # padded compacted box count

    # flat indices into the [80*13] box axis, frames in sorted order
    flat_idx = np.concatenate(
        [np.arange(t * _MAXN, t * _MAXN + int(n[t])) for t in order]
    )

    # pooling groups: runs of frames with equal box count n -> one strided
    # [128, cnt, n] max-reduce each. (frame_off, cnt, nval, col_off)
    groups = []
    col = 0
    f = 0
    for val in np.unique(ns):
        cnt = int((ns == val).sum())
        groups.append((f, cnt, int(val), col))
        f += cnt
        col += cnt * int(val)
    assert col == V and f == _BT

    # box-column chunks (matmul moving-operand tiles), each <=512, mult of 16
    nn = max(1, -(-Vp // 512))
    base = ((-(-Vp // nn)) + 15) // 16 * 16
    chunks = []
    off = 0
    while off < Vp:
        sz = min(base, Vp - off)
        chunks.append((off, sz))
        off += sz

    # contraction k-tiles per core (partition-dim tiles of <=128)
    kps = []
    ko = 0
    while ko < _KC:
        kp = min(128, _KC - ko)
        kps.append((ko, kp))
        ko += kp

    return n, order, flat_idx, V, Vp, groups, chunks, kps


def _build(Vp, groups, chunks, kps):
    """Build the SPMD bass program (identical on all 8 cores)."""
    import concourse.bass as bass
    import concourse.tile as tile
    from concourse import bacc, mybir

    f32 = mybir.dt.float32
    if _MM_DTYPE == "bf16":
        in_dt = mybir.dt.bfloat16
    elif _MM_DTYPE == "f32r":
        # fp32 bits; PE matmul runs at full (bf16) rate for N>=256. The BIR
        # verifier requires the matmul inputs to be *declared* float32r all
        # the way through (DRAM + SBUF), not bitcast at the consumer.
        in_dt = mybir.dt.float32r
    else:
        in_dt = f32

    nc = bacc.Bacc(
        "TRN2",
        target_bir_lowering=False,
        debug=False,
        enable_asserts=True,
        num_devices=_NC,
    )

    x_d = nc.dram_tensor("x", [_KC, Vp], in_dt, kind="ExternalInput")
    w_d = nc.dram_tensor("w", [_KC, _NFB], in_dt, kind="ExternalInput")
    be_d = nc.dram_tensor("be", [_FC, 1], f32, kind="ExternalInput")
    wa_d = nc.dram_tensor("wa", [_FC, _A], f32, kind="ExternalInput")
    wy_d = nc.dram_tensor("wy", [_FC, _G], f32, kind="ExternalInput")
    oa_d = nc.dram_tensor("out_act", [_A, Vp], f32, kind="ExternalOutput")
    oy_d = nc.dram_tensor("out_acty", [_G, _BT], f32, kind="ExternalOutput")
    hp_d = nc.dram_tensor("hpart", [_NFB, Vp], f32)                    # internal
    rs_d = nc.dram_tensor("rsout", [_FC, Vp], f32)

    KT = len(kps)

    def mm_ap(ap):
        return ap

    with tile.TileContext(nc) as tc:
        with (
            tc.tile_pool(name="sb", bufs=1) as sb,
            tc.tile_pool(name="psum", bufs=1, space="PSUM") as psum,
        ):
            # w_emb k-tiles: fully resident (each byte DMA'd exactly once)
            w_tiles = []
            for ko, kp in kps:
                wt = sb.tile([kp, _NFB], in_dt, tag="w", bufs=KT, name=f"w{ko}")
                nc.scalar.dma_start(wt[:], w_d[ko : ko + kp, :])
                w_tiles.append(wt)

            # stage 1: H^T partial [1024, Vp] = sum_k w[k,:]^T @ x[k,:]
            for co, nsz in chunks:
                xt = []
                for ki, (ko, kp) in enumerate(kps):
                    t = sb.tile([kp, nsz], in_dt, tag="x", bufs=KT + 4,
                                name=f"x{co}_{ko}")
                    nc.sync.dma_start(t[:], x_d[ko : ko + kp, co : co + nsz])
                    xt.append(t)
                for m in range(_NFB // 128):
                    ps = psum.tile([128, nsz], f32, tag="ps", bufs=3,
                                   name=f"ps{co}_{m}")
                    for ki in range(KT):
                        nc.tensor.matmul(
                            ps[:],
                            mm_ap(w_tiles[ki][:, m * 128 : (m + 1) * 128]),
                            mm_ap(xt[ki][:]),
                            start=(ki == 0),
                            stop=(ki == KT - 1),
                        )
                    st = sb.tile([128, nsz], f32, tag="st", bufs=4,
                                 name=f"st{co}_{m}")
                    nc.vector.tensor_copy(st[:], ps[:])
                    nc.gpsimd.dma_start(
                        hp_d[m * 128 : (m + 1) * 128, co : co + nsz], st[:]
                    )

            # sum partials across cores; core i keeps feature rows 128i..128i+128
            nc.gpsimd.collective_compute(
                "ReduceScatter",
                mybir.AluOpType.add,
                replica_groups=[list(range(_NC))],
                ins=[hp_d[:]],
                outs=[rs_d[:]],
            )

            # stage 2 (feature-sharded): bias+relu, actions, max-pool, activities
            h = sb.tile([_FC, Vp], f32, tag="h", bufs=1)
            nc.sync.dma_start(h[:], rs_d[:])
            bt = sb.tile([_FC, 1], f32, tag="bt", bufs=1)
            nc.sync.dma_start(bt[:], be_d[:])
            hr = sb.tile([_FC, Vp], f32, tag="hr", bufs=1)
            nc.scalar.activation(
                hr[:], h[:], mybir.ActivationFunctionType.Relu, bias=bt[:, 0:1]
            )

            wa = sb.tile([_FC, _A], f32, tag="wa", bufs=1)
            nc.sync.dma_start(wa[:], wa_d[:])
            oa_sb = sb.tile([_A, Vp], f32, tag="oasb", bufs=1)
            for co, nsz in chunks:
                pa = psum.tile([_A, nsz], f32, tag="pa", bufs=2, name=f"pa{co}")
                nc.tensor.matmul(
                    pa[:], wa[:], hr[:, co : co + nsz], start=True, stop=True
                )
                nc.vector.tensor_copy(oa_sb[:, co : co + nsz], pa[:])
            nc.sync.dma_start(oa_d[:], oa_sb[:])

            pooled = sb.tile([_FC, _BT], f32, tag="pl", bufs=1)
            for fo, cnt, nv, co in groups:
                src = hr[:, co : co + cnt * nv].rearrange("p (c n) -> p c n", n=nv)
                nc.vector.reduce_max(
                    pooled[:, fo : fo + cnt], src, axis=mybir.AxisListType.X
                )

            wy = sb.tile([_FC, _G], f32, tag="wy", bufs=1)
            nc.sync.dma_start(wy[:], wy_d[:])
            py = psum.tile([_G, _BT], f32, tag="py", bufs=1)
            nc.tensor.matmul(py[:], wy[:], pooled[:], start=True, stop=True)
            oy_sb = sb.tile([_G, _BT], f32, tag="oysb", bufs=1)
            nc.vector.tensor_copy(oy_sb[:], py[:])
            nc.sync.dma_start(oy_d[:], oy_sb[:])

    nc.compile()
    return nc


def _run(inputs, trace=False, trace_kwargs=None):
    """Shard, run on 8 cores, gather. Returns (actions, activities, results)."""
    from concourse.bass_utils import run_bass_kernel_spmd

    boxes_features_flat = np.asarray(inputs["boxes_features_flat"], np.float32)
    w_emb = np.asarray(inputs["w_emb"], np.float32)
    b_emb = np.asarray(inputs["b_emb"], np.float32)
    w_act = np.asarray(inputs["w_act"], np.float32)
    b_act = np.asarray(inputs["b_act"], np.float32)
    w_acty = np.asarray(inputs["w_acty"], np.float32)
    b_acty = np.asarray(inputs["b_acty"], np.float32)
    bboxes_num = np.asarray(inputs["bboxes_num"])

    n, order, flat_idx, V, Vp, groups, chunks, kps = _plan(bboxes_num)

    # host marshalling: compact + transpose X to [K2D, Vp]
    X = boxes_features_flat.reshape(_BT * _MAXN, _K2D)
    XT = np.zeros((_K2D, Vp), np.float32)
    XT[:, :V] = X[flat_idx].T

    if _MM_DTYPE == "bf16":
        import ml_dtypes

        XT = XT.astype(ml_dtypes.bfloat16)
        w_in = w_emb.astype(ml_dtypes.bfloat16)
    else:
        w_in = w_emb

    in_maps = []
    for i in range(_NC):
        in_maps.append(
            {
                "x": np.ascontiguousarray(XT[_KC * i : _KC * (i + 1)]),
                "w": np.ascontiguousarray(w_in[_KC * i : _KC * (i + 1)]),
                "be": np.ascontiguousarray(
                    b_emb[_FC * i : _FC * (i + 1)].reshape(_FC, 1)
                ),
                "wa": np.ascontiguousarray(w_act[_FC * i : _FC * (i + 1)]),
                "wy": np.ascontiguousarray(w_acty[_FC * i : _FC * (i + 1)]),
            }
        )

    nc = _build(Vp, groups, chunks, kps)
    res = run_bass_kernel_spmd(
        nc,
        in_maps,
        list(range(_NC)),
        trace=trace,
        **(trace_kwargs or {}),
    )

    # gather: sum feature-shard partials, add biases, scatter to ragged layout
    act_T = np.zeros((_A, Vp), np.float32)
    acty_T = np.zeros((_G, _BT), np.float32)
    for i in range(_NC):
        act_T += res.results[i]["out_act"]
        acty_T += res.results[i]["out_acty"]

    actions = np.zeros((_BT * _MAXN, _A), np.float32)
    actions[flat_idx] = act_T[:, :V].T + b_act[None, :]
    actions = actions.reshape(_BT, _MAXN, _A)

    activities = np.zeros((_BT, _G), np.float32)
    activities[order] = acty_T.T + b_acty[None, :]

    return actions, activities, res


def kernel(**inputs):
    actions, activities, _ = _run(inputs, trace=False)
    return actions, activities


# revision 6
# speedup vs baseline: 1.5287x; 1.5287x over previous
"""Trainium2 Bass kernel for nn_Basenet_collective (ragged RoI-box MLP head).

Computes, for X=[80,13,26400] box features with ragged per-frame validity:
    h = relu(X @ w_emb + b_emb)                      [80,13,1024]
    actions = (h @ w_act + b_act) * valid_mask       [80,13,6]
    activities = max_pool_valid(h) @ w_acty + b_acty [80,5]

Distribution over 8 NeuronCores (one trn2 chip):
  - Host compacts the ragged box axis (only sum(bboxes_num) of the 80*13
    box slots contribute to the output), groups frames by box count, and
    transposes X to [K2D, V].
  - The 26400-deep contraction is split 8 ways (3300 per core); each core
    computes a partial H^T = w_emb_slice^T @ X_slice^T of shape [1024, V]
    on the tensor engine (fp32 PSUM accumulation).
  - Box columns are processed in a few chunks; each chunk's partial is
    ReduceScatter(add)-ed across cores as soon as it is ready, so the
    collectives overlap the remaining matmul work. After the last RS,
    core i holds feature rows [128i:128(i+1)] of the summed H^T.
  - Stage 2 (bias+relu, action scores, grouped max-pool, activity scores)
    is feature-sharded and frame-local on the device.
  - Host sums the 8 feature-shard partials of the two small outputs,
    adds biases, and scatters back to the original ragged layout.
"""

import numpy as np

_BT = 80
_MAXN = 13
_K2D = 26400
_NFB = 1024
_A = 6
_G = 5
_NC = 8
_KC = _K2D // _NC   # 3300 contraction rows per core
_FC = _NFB // _NC   # 128 feature rows per core after ReduceScatter

_MM_DTYPE = "bf16"  # 'bf16' | 'f32r' | 'f32' — stage-1 matmul input dtype
_CC_DTYPE = "f16"   # 'f16' | 'f32' — ReduceScatter payload dtype


def _chunk_cols(Vp):
    """Split Vp box columns into matmul/RS chunks: each <=512, mult of 16,
    mildly front-loaded so trailing ReduceScatters hide under compute."""
    if Vp <= 256:
        return [(0, Vp)]
    n = max(2, -(-Vp // 512))
    fracs = {2: [0.60, 0.40], 3: [0.45, 0.33, 0.22]}.get(
        n, [2.0 * (n - i) / (n * (n + 1)) for i in range(n)]
    )
    sizes = [min(512, max(16, int(Vp * f / 16) * 16)) for f in fracs]
    sizes[-1] += Vp - sum(sizes)
    while sizes[-1] > 512:  # push overflow forward
        for i in range(n - 1):
            take = min(512 - sizes[i], sizes[-1] - 512)
            sizes[i] += take
            sizes[-1] -= take
            if sizes[-1] <= 512:
                break
    assert sum(sizes) == Vp and all(0 < s <= 512 for s in sizes), sizes
    chunks = []
    off = 0
    for s in sizes:
        chunks.append((off, s))
        off += s
    return chunks


def _plan(bboxes_num):
    """Host-side plan: compaction order, pooling groups, column chunks."""
    n = np.asarray(bboxes_num).astype(np.int64)
    assert n.shape == (_BT,) and n.min() >= 1 and n.max() <= _MAXN
    order = np.argsort(n, kind="stable")          # frames sorted by box count
    ns = n[order]
    V = int(n.sum())
    Vp = ((max(V, 64) + 15) // 16) * 16           # padded compacted box count

    # flat indices into the [80*13] box axis, frames in sorted order
    flat_idx = np.concatenate(
        [np.arange(t * _MAXN, t * _MAXN + int(n[t])) for t in order]
    )

    # pooling groups: runs of frames with equal box count n -> one strided
    # [128, cnt, n] max-reduce each. (frame_off, cnt, nval, col_off)
    groups = []
    col = 0
    f = 0
    for val in np.unique(ns):
        cnt = int((ns == val).sum())
        groups.append((f, cnt, int(val), col))
        f += cnt
        col += cnt * int(val)
    assert col == V and f == _BT

    chunks = _chunk_cols(Vp)

    # contraction k-tiles per core (partition-dim tiles of <=128)
    kps = []
    ko = 0
    while ko < _KC:
        kp = min(128, _KC - ko)
        kps.append((ko, kp))
        ko += kp

    return n, order, flat_idx, V, Vp, groups, chunks, kps


def _build(Vp, groups, chunks, kps):
    """Build the SPMD bass program (identical on all 8 cores)."""
    import concourse.bass as bass
    import concourse.tile as tile
    from concourse import bacc, mybir

    f32 = mybir.dt.float32
    if _MM_DTYPE == "bf16":
        in_dt = mybir.dt.bfloat16
    elif _MM_DTYPE == "f32r":
        # fp32 bits; PE matmul runs at full (bf16) rate for N>=256. The BIR
        # verifier requires the matmul inputs to be *declared* float32r all
        # the way through (DRAM + SBUF), not bitcast at the consumer.
        in_dt = mybir.dt.float32r
    else:
        in_dt = f32
    cc_dt = mybir.dt.float16 if _CC_DTYPE == "f16" else f32

    nc = bacc.Bacc(
        "TRN2",
        target_bir_lowering=False,
        debug=False,
        enable_asserts=True,
        num_devices=_NC,
    )

    x_d = nc.dram_tensor("x", [_KC, Vp], in_dt, kind="ExternalInput")
    w_d = nc.dram_tensor("w", [_KC, _NFB], in_dt, kind="ExternalInput")
    be_d = nc.dram_tensor("be", [_FC, 1], f32, kind="ExternalInput")
    wa_d = nc.dram_tensor("wa", [_FC, _A], f32, kind="ExternalInput")
    wy_d = nc.dram_tensor("wy", [_FC, _G], f32, kind="ExternalInput")
    oa_d = nc.dram_tensor("out_act", [_A, Vp], f32, kind="ExternalOutput")
    oy_d = nc.dram_tensor("out_acty", [_G, _BT], f32, kind="ExternalOutput")
    # per-chunk collective bounce buffers (internal DRAM)
    hp_d = [
        nc.dram_tensor(f"hpart{ci}", [_NFB, nsz], cc_dt)
        for ci, (_, nsz) in enumerate(chunks)
    ]
    rs_d = [
        nc.dram_tensor(f"rsout{ci}", [_FC, nsz], cc_dt)
        for ci, (_, nsz) in enumerate(chunks)
    ]

    KT = len(kps)

    with tile.TileContext(nc) as tc:
        with (
            tc.tile_pool(name="sb", bufs=1) as sb,
            tc.tile_pool(name="psum", bufs=1, space="PSUM") as psum,
        ):
            # w_emb k-tiles: fully resident (each byte DMA'd exactly once)
            w_tiles = []
            for ko, kp in kps:
                wt = sb.tile([kp, _NFB], in_dt, tag="w", bufs=KT, name=f"w{ko}")
                nc.scalar.dma_start(wt[:], w_d[ko : ko + kp, :])
                w_tiles.append(wt)

            # stage 1: per column chunk, H^T partial = sum_k w[k]^T @ x[k],
            # k-outer with two m-half passes so the PE never waits on the
            # full weight sweep and 8 PSUM banks suffice.
            for ci, (co, nsz) in enumerate(chunks):
                xt = []
                for ko, kp in kps:
                    t = sb.tile([kp, nsz], in_dt, tag="x", bufs=KT + 4,
                                name=f"x{ci}_{ko}")
                    nc.sync.dma_start(t[:], x_d[ko : ko + kp, co : co + nsz])
                    xt.append(t)
                for half in range(2):
                    ps = [
                        psum.tile([128, nsz], f32, tag="ps", bufs=8,
                                  name=f"ps{ci}_{half}_{m4}")
                        for m4 in range(4)
                    ]
                    for ki in range(KT):
                        for m4 in range(4):
                            m = half * 4 + m4
                            nc.tensor.matmul(
                                ps[m4][:],
                                w_tiles[ki][:, m * 128 : (m + 1) * 128],
                                xt[ki][:],
                                start=(ki == 0),
                                stop=(ki == KT - 1),
                            )
                    for m4 in range(4):
                        m = half * 4 + m4
                        st = sb.tile([128, nsz], cc_dt, tag="st", bufs=6,
                                     name=f"st{ci}_{m}")
                        nc.vector.tensor_copy(st[:], ps[m4][:])
                        nc.gpsimd.dma_start(
                            hp_d[ci][m * 128 : (m + 1) * 128, :], st[:]
                        )
                # chunk's partial is complete: reduce+scatter it across cores
                # while later chunks keep the PE busy
                nc.gpsimd.collective_compute(
                    "ReduceScatter",
                    mybir.AluOpType.add,
                    replica_groups=[list(range(_NC))],
                    ins=[hp_d[ci][:]],
                    outs=[rs_d[ci][:]],
                )

            # stage 2 (feature-sharded): bias+relu per chunk as its RS lands
            bt = sb.tile([_FC, 1], f32, tag="bt", bufs=1)
            nc.sync.dma_start(bt[:], be_d[:])
            wa = sb.tile([_FC, _A], f32, tag="wa", bufs=1)
            nc.sync.dma_start(wa[:], wa_d[:])
            wy = sb.tile([_FC, _G], f32, tag="wy", bufs=1)
            nc.sync.dma_start(wy[:], wy_d[:])

            hr = sb.tile([_FC, Vp], f32, tag="hr", bufs=1)
            oa_sb = sb.tile([_A, Vp], f32, tag="oasb", bufs=1)
            for ci, (co, nsz) in enumerate(chunks):
                h2 = sb.tile([_FC, nsz], cc_dt, tag="h2", bufs=2, name=f"h2_{ci}")
                nc.sync.dma_start(h2[:], rs_d[ci][:])
                nc.scalar.activation(
                    hr[:, co : co + nsz],
                    h2[:],
                    mybir.ActivationFunctionType.Relu,
                    bias=bt[:, 0:1],
                )
                pa = psum.tile([_A, nsz], f32, tag="ps", bufs=8, name=f"pa{ci}")
                nc.tensor.matmul(
                    pa[:], wa[:], hr[:, co : co + nsz], start=True, stop=True
                )
                nc.vector.tensor_copy(oa_sb[:, co : co + nsz], pa[:])
            nc.sync.dma_start(oa_d[:], oa_sb[:])

            # masked max-pool: one strided reduce per group of equal box count
            pooled = sb.tile([_FC, _BT], f32, tag="pl", bufs=1)
            for fo, cnt, nv, co in groups:
                src = hr[:, co : co + cnt * nv].rearrange("p (c n) -> p c n", n=nv)
                nc.vector.reduce_max(
                    pooled[:, fo : fo + cnt], src, axis=mybir.AxisListType.X
                )

            py = psum.tile([_G, _BT], f32, tag="ps", bufs=8, name="py")
            nc.tensor.matmul(py[:], wy[:], pooled[:], start=True, stop=True)
            oy_sb = sb.tile([_G, _BT], f32, tag="oysb", bufs=1)
            nc.vector.tensor_copy(oy_sb[:], py[:])
            nc.sync.dma_start(oy_d[:], oy_sb[:])

    nc.compile()
    return nc


def _run(inputs, trace=False, trace_kwargs=None):
    """Shard, run on 8 cores, gather. Returns (actions, activities, results)."""
    from concourse.bass_utils import run_bass_kernel_spmd

    boxes_features_flat = np.asarray(inputs["boxes_features_flat"], np.float32)
    w_emb = np.asarray(inputs["w_emb"], np.float32)
    b_emb = np.asarray(inputs["b_emb"], np.float32)
    w_act = np.asarray(inputs["w_act"], np.float32)
    b_act = np.asarray(inputs["b_act"], np.float32)
    w_acty = np.asarray(inputs["w_acty"], np.float32)
    b_acty = np.asarray(inputs["b_acty"], np.float32)
    bboxes_num = np.asarray(inputs["bboxes_num"])

    n, order, flat_idx, V, Vp, groups, chunks, kps = _plan(bboxes_num)

    # host marshalling: compact + transpose X to [K2D, Vp]
    X = boxes_features_flat.reshape(_BT * _MAXN, _K2D)
    XT = np.zeros((_K2D, Vp), np.float32)
    XT[:, :V] = X[flat_idx].T

    if _MM_DTYPE == "bf16":
        import ml_dtypes

        XT = XT.astype(ml_dtypes.bfloat16)
        w_in = w_emb.astype(ml_dtypes.bfloat16)
    else:
        w_in = w_emb

    in_maps = []
    for i in range(_NC):
        in_maps.append(
            {
                "x": np.ascontiguousarray(XT[_KC * i : _KC * (i + 1)]),
                "w": np.ascontiguousarray(w_in[_KC * i : _KC * (i + 1)]),
                "be": np.ascontiguousarray(
                    b_emb[_FC * i : _FC * (i + 1)].reshape(_FC, 1)
                ),
                "wa": np.ascontiguousarray(w_act[_FC * i : _FC * (i + 1)]),
                "wy": np.ascontiguousarray(w_acty[_FC * i : _FC * (i + 1)]),
            }
        )

    nc = _build(Vp, groups, chunks, kps)
    res = run_bass_kernel_spmd(
        nc,
        in_maps,
        list(range(_NC)),
        trace=trace,
        **(trace_kwargs or {}),
    )

    # gather: sum feature-shard partials, add biases, scatter to ragged layout
    act_T = np.zeros((_A, Vp), np.float32)
    acty_T = np.zeros((_G, _BT), np.float32)
    for i in range(_NC):
        act_T += res.results[i]["out_act"]
        acty_T += res.results[i]["out_acty"]

    actions = np.zeros((_BT * _MAXN, _A), np.float32)
    actions[flat_idx] = act_T[:, :V].T + b_act[None, :]
    actions = actions.reshape(_BT, _MAXN, _A)

    activities = np.zeros((_BT, _G), np.float32)
    activities[order] = acty_T.T + b_acty[None, :]

    return actions, activities, res


def kernel(**inputs):
    actions, activities, _ = _run(inputs, trace=False)
    return actions, activities
